# revision 8
# baseline (speedup 1.0000x reference)
"""BiDAF Trainium2 Bass kernel — data-parallel over batch across 8 NeuronCores.

Layouts: feature-major activations (features on partitions, tokens on free).
LSTM recurrence: weights-stationary matmuls (lhsT = W_hh^T tiles), merged
fwd/bwd elementwise chains. Gate order on device: i, f, o, g (128-chunks).
"""
import sys
sys.path.insert(0, '/opt/trn_rl_repo')

from contextlib import ExitStack

import numpy as np
import ml_dtypes

import concourse.bass as bass
import concourse.mybir as mybir
import concourse.tile as tile
from concourse import bacc
from concourse.bass_utils import run_bass_kernel_spmd
from concourse.masks import make_identity

dt = mybir.dt
AF = mybir.ActivationFunctionType
ALU = mybir.AluOpType
BF16 = ml_dtypes.bfloat16

H = 256
B = 32
NCORES = 8
BC = B // NCORES          # 4 sequences per core
T = 512                   # context length
TQ = 64                   # query length
NTOK = BC * T + BC * TQ   # 2304 tokens per core (ctx then q)
NCTX = BC * T
WLEN = 16
CCS = 2 * H - 50          # 462
NCHARS = NTOK * WLEN      # 36864

CHUNKS = [(0, 512), (512, 1024), (1024, 1536), (1536, 2048), (2048, 2304)]
CTX_CHUNKS = CHUNKS[:4]
Q_CHUNKS = [(2048, 2304)]

GATE_PERM = np.concatenate([np.arange(0, 512), np.arange(768, 1024), np.arange(512, 768)])
ABLATE = set()  # perf-analysis ablations: subsets of {"rec","ip","char","hw","attn","heads"}


# --------------------------------------------------------------------------
# host-side parameter preprocessing
# --------------------------------------------------------------------------

def _prep_params(p):
    out = {}
    f32 = np.float32

    ce = np.zeros((384, 16), f32)
    ce[:262] = np.asarray(p['char_emb'], f32)
    out['char_emb_t'] = np.ascontiguousarray(ce.reshape(3, 128, 16).transpose(1, 0, 2)).astype(BF16)

    cw = np.asarray(p['conv_w'], f32)  # (462,1,16,5)
    convw = np.zeros((12, 256, CCS), f32)
    for t in range(12):
        for k in range(5):
            u = t + k
            convw[t, np.arange(16) * 16 + u, :] = cw[:, 0, :, k].T
    out['conv_w_t'] = np.ascontiguousarray(
        convw.reshape(12, 2, 128, CCS).transpose(2, 0, 1, 3)).astype(BF16)  # (128,12,2,462)
    cb = np.zeros((128, 4), f32)
    cbv = np.asarray(p['conv_b'], f32)
    cb[:, :3] = cbv[:384].reshape(3, 128).T
    cb[:CCS - 384, 3] = cbv[384:]
    out['conv_b'] = cb

    out['word_emb'] = np.asarray(p['word_emb'], f32)

    for i in range(2):
        for nm, wkey, bkey in (("lin", f'hw_lin_w{i}', f'hw_lin_b{i}'),
                               ("gate", f'hw_gate_w{i}', f'hw_gate_b{i}')):
            w = np.asarray(p[wkey], f32)
            out[f'hw_{nm}{i}_t'] = np.ascontiguousarray(
                w.T.reshape(4, 128, 512).transpose(1, 0, 2)).astype(BF16)  # (128,4,512)
            out[f'hw_{nm}{i}_b'] = np.ascontiguousarray(np.asarray(p[bkey], f32).reshape(4, 128).T)

    def lstm(prefix, pr, in_dim):
        kt = in_dim // 128
        for d, (wi, wh, bb) in (("f", ('wif', 'whf', 'bf')), ("b", ('wib', 'whb', 'bb'))):
            wid = np.asarray(pr[wi], f32)[GATE_PERM]        # (1024, in)
            out[f'{prefix}_ip_{d}'] = np.ascontiguousarray(
                wid.T.reshape(kt, 128, 1024).transpose(1, 0, 2)).astype(BF16)  # (128,kt,1024)
            out[f'{prefix}_ipb_{d}'] = np.ascontiguousarray(
                np.asarray(pr[bb], f32)[GATE_PERM].reshape(8, 128).T)
            whd = np.asarray(pr[wh], f32)[GATE_PERM]        # (1024, 256)
            lt = np.zeros((128, 16, 128), f32)
            for k in range(2):
                for m in range(8):
                    lt[:, k * 8 + m, :] = whd[m * 128:(m + 1) * 128, k * 128:(k + 1) * 128].T
            out[f'{prefix}_whh_{d}'] = lt.astype(BF16)

    lstm('ctx', p['ctx'], 512)
    lstm('mod1', p['mod1'], 2048)
    lstm('mod2', p['mod2'], 512)
    lstm('out', p['out'], 512)

    out['att_wc'] = np.ascontiguousarray(np.asarray(p['w_c'], f32).reshape(4, 128).T)
    out['att_wcq'] = np.ascontiguousarray(np.asarray(p['w_cq'], f32).reshape(4, 128).T)
    out['att_wq'] = np.ascontiguousarray(np.asarray(p['w_q'], f32).reshape(4, 128).T).astype(BF16)
    out['att_b'] = np.array([[float(p['b_c']) + float(p['b_q']) + float(p['b_cq'])]], f32)

    out['p1_gw'] = np.ascontiguousarray(np.asarray(p['p1_g_w'], f32).reshape(16, 128).T).astype(BF16)
    out['p1_mw'] = np.ascontiguousarray(np.asarray(p['p1_m_w'], f32).reshape(4, 128).T).astype(BF16)
    out['p1_b'] = np.array([[float(p['p1_g_b']) + float(p['p1_m_b'])]], f32)
    out['p2_gw'] = np.ascontiguousarray(np.asarray(p['p2_g_w'], f32).reshape(16, 128).T).astype(BF16)
    out['p2_mw'] = np.ascontiguousarray(np.asarray(p['p2_m_w'], f32).reshape(4, 128).T).astype(BF16)
    out['p2_b'] = np.array([[float(p['p2_g_b']) + float(p['p2_m_b'])]], f32)
    return out


def _prep_core_inputs(prep, c_word, q_word, c_char, q_char, core):
    sl = slice(core * BC, (core + 1) * BC)
    ins = dict(prep)
    tok = np.concatenate([np.asarray(c_word)[sl].reshape(-1),
                          np.asarray(q_word)[sl].reshape(-1)]).astype(np.int32)
    ins['tok_idx'] = np.ascontiguousarray(tok[:, None])
    ch = np.concatenate([np.asarray(c_char)[sl].reshape(NCTX, WLEN),
                         np.asarray(q_char)[sl].reshape(BC * TQ, WLEN)])
    ins['char_ids'] = np.ascontiguousarray(ch.T.reshape(1, NCHARS)).astype(np.float16)
    return ins


# --------------------------------------------------------------------------
# device program pieces
# --------------------------------------------------------------------------

def _emit_bilstm(nc, psum_pool, state_pool, Tn, xp_f, xp_b, whh_f, whh_b, hseq, h0):
    """Merged-direction LSTM. xp_*: (128, 8, BC, Tn) bf16 views.
    whh_*: (128, 16, 128) bf16. hseq: (128, 2, 2*BC, Tn) bf16. h0: zeros."""
    if "rec" in ABLATE:
        nc.vector.memset(hseq[:], 0.0)
        return
    c_st = state_pool.tile([128, 2, 2 * BC], dt.float32, tag="c_st")
    nc.vector.memset(c_st[:], 0.0)
    S = state_pool.tile([128, 8, 2 * BC], dt.float32, tag="S")
    t1 = state_pool.tile([128, 2, 2 * BC], dt.float32, tag="t1")
    t2 = state_pool.tile([128, 2, 2 * BC], dt.float32, tag="t2")
    TC = state_pool.tile([128, 2, 2 * BC], dt.float32, tag="TC")
    A = state_pool.tile([128, 6, 2 * BC], dt.float32, tag="A")
    TG = state_pool.tile([128, 2, 2 * BC], dt.float32, tag="TG")
    h_cur = state_pool.tile([128, 2, 2 * BC], dt.bfloat16, tag="h_cur")

    ident = state_pool.tile([128, 128], dt.bfloat16, tag="rec_ident")
    make_identity(nc, ident[:])
    for t in range(Tn):
        tb = Tn - 1 - t
        G = psum_pool.tile([128, 8, 2 * BC], dt.float32, tag="G", bufs=2)
        last = (t == 0)
        for m in range(8):
            nc.tensor.matmul(G[:, m, 0:BC], lhsT=ident[:], rhs=xp_f[:, m, :, t],
                             start=True, stop=last, skip_group_check=True)
            nc.tensor.matmul(G[:, m, BC:], lhsT=ident[:], rhs=xp_b[:, m, :, tb],
                             start=True, stop=last, skip_group_check=True)
        if t > 0:
            hf, hb = h_cur[:, :, 0:BC], h_cur[:, :, BC:]
            for m in range(8):
                nc.tensor.matmul(G[:, m, 0:BC], lhsT=whh_f[:, m, :], rhs=hf[:, 0, :],
                                 start=False, stop=False, skip_group_check=True)
                nc.tensor.matmul(G[:, m, 0:BC], lhsT=whh_f[:, 8 + m, :], rhs=hf[:, 1, :],
                                 start=False, stop=True, skip_group_check=True)
            for m in range(8):
                nc.tensor.matmul(G[:, m, BC:], lhsT=whh_b[:, m, :], rhs=hb[:, 0, :],
                                 start=False, stop=False, skip_group_check=True)
                nc.tensor.matmul(G[:, m, BC:], lhsT=whh_b[:, 8 + m, :], rhs=hb[:, 1, :],
                                 start=False, stop=True, skip_group_check=True)
        nc.scalar.activation(out=A[:], in_=G[:, 0:6, :], func=AF.Sigmoid)
        nc.scalar.activation(out=TG[:], in_=G[:, 6:8, :], func=AF.Tanh)
        nc.vector.tensor_tensor(out=t1[:], in0=A[:, 2:4, :], in1=c_st[:], op=ALU.mult)
        nc.vector.tensor_tensor(out=t2[:], in0=A[:, 0:2, :], in1=TG[:], op=ALU.mult)
        nc.vector.tensor_tensor(out=c_st[:], in0=t1[:], in1=t2[:], op=ALU.add)
        nc.scalar.activation(out=TC[:], in_=c_st[:], func=AF.Tanh)
        nc.vector.tensor_tensor(out=h_cur[:], in0=A[:, 4:6, :], in1=TC[:], op=ALU.mult)
        nc.gpsimd.tensor_copy(out=hseq[:, :, 0:BC, t], in_=h_cur[:, :, 0:BC])
        nc.gpsimd.tensor_copy(out=hseq[:, :, BC:, tb], in_=h_cur[:, :, BC:])


def _emit_ip(nc, psum_pool, rhs_tiles, w_t, bias, xp, chunks, tok0):
    """xp[:, m, cols] = sum_k w_t[:, k, m*128:(m+1)*128].T @ rhs_k[:, chunk] + bias."""
    if "ip" in ABLATE:
        nc.vector.memset(xp[:], 0.0)
        return
    for (c0, c1) in chunks:
        n = c1 - c0
        for m in range(8):
            ps = psum_pool.tile([128, 512], dt.float32, tag="ip", bufs=4)
            nk = len(rhs_tiles)
            for k in range(nk):
                nc.tensor.matmul(ps[:, 0:n], lhsT=w_t[:, k, m * 128:(m + 1) * 128],
                                 rhs=rhs_tiles[k][:, c0:c1], start=(k == 0), stop=(k == nk - 1))
            nc.scalar.activation(out=xp[:, m, c0 - tok0:c1 - tok0], in_=ps[:, 0:n],
                                 func=AF.Identity, bias=bias[:, m:m + 1])


def build_program(debug=False):
    nc = bacc.Bacc("TRN2", target_bir_lowering=False, debug=False)

    d_tok = nc.dram_tensor("tok_idx", [NTOK, 1], dt.int32, kind="ExternalInput")
    d_chars = nc.dram_tensor("char_ids", [1, NCHARS], dt.float16, kind="ExternalInput")
    d_cemb = nc.dram_tensor("char_emb_t", [128, 3, 16], dt.bfloat16, kind="ExternalInput")
    d_convw = nc.dram_tensor("conv_w_t", [128, 12, 2, CCS], dt.bfloat16, kind="ExternalInput")
    d_convb = nc.dram_tensor("conv_b", [128, 4], dt.float32, kind="ExternalInput")
    d_wemb = nc.dram_tensor("word_emb", [50000, 50], dt.float32, kind="ExternalInput")
    d_hw = {}
    for i in range(2):
        for nm in ("lin", "gate"):
            d_hw[f'{nm}{i}'] = nc.dram_tensor(f"hw_{nm}{i}_t", [128, 4, 512], dt.bfloat16, kind="ExternalInput")
            d_hw[f'{nm}{i}_b'] = nc.dram_tensor(f"hw_{nm}{i}_b", [128, 4], dt.float32, kind="ExternalInput")
    d_lstm = {}
    for st, ind in (('ctx', 512), ('mod1', 2048), ('mod2', 512), ('out', 512)):
        for d in ('f', 'b'):
            d_lstm[f'{st}_ip_{d}'] = nc.dram_tensor(f"{st}_ip_{d}", [128, ind // 128, 1024], dt.bfloat16, kind="ExternalInput")
            d_lstm[f'{st}_ipb_{d}'] = nc.dram_tensor(f"{st}_ipb_{d}", [128, 8], dt.float32, kind="ExternalInput")
            d_lstm[f'{st}_whh_{d}'] = nc.dram_tensor(f"{st}_whh_{d}", [128, 16, 128], dt.bfloat16, kind="ExternalInput")
    d_awc = nc.dram_tensor("att_wc", [128, 4], dt.float32, kind="ExternalInput")
    d_awcq = nc.dram_tensor("att_wcq", [128, 4], dt.float32, kind="ExternalInput")
    d_awq = nc.dram_tensor("att_wq", [128, 4], dt.bfloat16, kind="ExternalInput")
    d_ab = nc.dram_tensor("att_b", [1, 1], dt.float32, kind="ExternalInput")
    d_heads = {}
    for nm, sh, dd in (("p1_gw", [128, 16], dt.bfloat16), ("p1_mw", [128, 4], dt.bfloat16),
                       ("p2_gw", [128, 16], dt.bfloat16), ("p2_mw", [128, 4], dt.bfloat16),
                       ("p1_b", [1, 1], dt.float32), ("p2_b", [1, 1], dt.float32)):
        d_heads[nm] = nc.dram_tensor(nm, sh, dd, kind="ExternalInput")

    d_p1 = nc.dram_tensor("p1", [1, NCTX], dt.float32, kind="ExternalOutput")
    d_p2 = nc.dram_tensor("p2", [1, NCTX], dt.float32, kind="ExternalOutput")

    gkind = "ExternalOutput" if debug else "Internal"
    d_g = nc.dram_tensor("g_buf", [16, 128, NCTX], dt.bfloat16, kind=gkind)
    if debug:
        d_xhw = nc.dram_tensor("x_hw", [4, 128, NTOK], dt.bfloat16, kind="ExternalOutput")
        d_cenc = nc.dram_tensor("c_enc", [128, 2, 2 * BC, T], dt.bfloat16, kind="ExternalOutput")
        d_qenc = nc.dram_tensor("q_enc", [128, 2, 2 * BC, TQ], dt.bfloat16, kind="ExternalOutput")
        d_m1 = nc.dram_tensor("m1_seq", [128, 2, 2 * BC, T], dt.bfloat16, kind="ExternalOutput")
        d_m2 = nc.dram_tensor("m2_seq", [128, 2, 2 * BC, T], dt.bfloat16, kind="ExternalOutput")
        d_xpdbg = nc.dram_tensor("xp_dbg", [128, 8, NCTX], dt.bfloat16, kind="ExternalOutput")

    es = ExitStack()
    with tile.TileContext(nc) as tc:
        const = es.enter_context(tc.tile_pool(name="const", bufs=1))
        ident_bf = const.tile([128, 128], dt.bfloat16)
        make_identity(nc, ident_bf[:])
        ident_f = const.tile([128, 128], dt.float32)
        make_identity(nc, ident_f[:])
        ones_row_bf = const.tile([1, 128], dt.bfloat16)
        nc.vector.memset(ones_row_bf[:], 1.0)
        ones_col = const.tile([128, 1], dt.float32)
        nc.vector.memset(ones_col[:], 1.0)
        ones_row_f = const.tile([1, 128], dt.float32)
        nc.vector.memset(ones_row_f[:], 1.0)
        iota3 = const.tile([128, 3], dt.float32)
        iota_i = const.tile([128, 1], dt.int32)
        nc.gpsimd.iota(iota_i[:], [[0, 1]], channel_multiplier=1)
        for v in range(3):
            nc.vector.tensor_scalar(out=iota3[:, v:v + 1], in0=iota_i[:], scalar1=float(v * 128),
                                    scalar2=None, op0=ALU.add)

        seq_pool = es.enter_context(tc.tile_pool(name="seq", bufs=1))
        enc_es = ExitStack()
        enc_pool = enc_es.enter_context(tc.tile_pool(name="encp", bufs=1))
        hseq_c = enc_pool.tile([128, 2, 2 * BC, T], dt.bfloat16, tag="hseq_c")
        hseq_q = enc_pool.tile([128, 2, 2 * BC, TQ], dt.bfloat16, tag="hseq_q")

        with tc.tile_pool(name="xt", bufs=1) as xtile_pool:
            xs = [[xtile_pool.tile([128, NTOK], dt.bfloat16, tag=f"x{st}{m}", name=f"x{st}{m}") for m in range(4)]
                  for st in range(2)]
            xt = xs[0]

            # =========== PHASE A: char conv + word emb ===========
            with tc.tile_pool(name="chA", bufs=1) as cp, \
                 tc.tile_pool(name="chB", bufs=2) as cb_, \
                 tc.tile_pool(name="psA", bufs=2, space="PSUM") as pp:
                cet = cp.tile([128, 3, 16], dt.bfloat16)
                nc.sync.dma_start(out=cet[:], in_=d_cemb.ap())
                XT = [cp.tile([128, NTOK], dt.bfloat16, tag=f"XT{i}", name=f"XT{i}") for i in range(2)]
                NCH = NTOK // 2  # half char-slot per chunk
                for ch in range(32):
                    ids_b = cb_.tile([128, NCH], dt.float16, tag="ids")
                    bap = bass.AP(tensor=d_chars.ap().tensor, offset=ch * NCH, ap=[[0, 128], [1, NCH]])
                    nc.sync.dma_start(out=ids_b[:], in_=bap)
                    ohs = cb_.tile([128, 3, NCH], dt.bfloat16, tag="oh")
                    for v in range(3):
                        nc.vector.tensor_scalar(out=ohs[:, v, :], in0=ids_b[:], scalar1=iota3[:, v:v + 1],
                                                scalar2=None, op0=ALU.is_equal)
                    Et = cb_.tile([16, NCH], dt.bfloat16, tag="Et")
                    for si in range((NCH + 511) // 512):
                        s0 = si * 512
                        s1 = min(s0 + 512, NCH)
                        ps = pp.tile([16, 512], dt.float32, tag="E")
                        for v in range(3):
                            nc.tensor.matmul(ps[:, 0:s1 - s0], lhsT=cet[:, v, :], rhs=ohs[:, v, s0:s1],
                                             start=(v == 0), stop=(v == 2))
                        nc.scalar.activation(out=Et[:, s0:s1], in_=ps[:, 0:s1 - s0], func=AF.Copy)
                    slot = ch // 2
                    half = (ch % 2) * NCH
                    nc.sync.dma_start(out=XT[slot // 8][(slot % 8) * 16:(slot % 8) * 16 + 16, half:half + NCH],
                                      in_=Et[:])
                convw = cp.tile([128, 12, 2, CCS], dt.bfloat16)
                nc.sync.dma_start(out=convw[:], in_=d_convw.ap())
                convb = cp.tile([128, 4], dt.float32)
                nc.sync.dma_start(out=convb[:], in_=d_convb.ap())
                for mi in range(4):
                    mr = 128 if mi < 3 else CCS - 384
                    for (c0, c1) in CHUNKS:
                        n = c1 - c0
                        shifts = cb_.tile([128, 512, 12], dt.bfloat16, tag="shifts")
                        for t in range(12):
                            ps = pp.tile([128, 512], dt.float32, tag="conv")
                            for k in range(2):
                                nc.tensor.matmul(ps[0:mr, 0:n], lhsT=convw[:, t, k, mi * 128:mi * 128 + mr],
                                                 rhs=XT[k][:, c0:c1], start=(k == 0), stop=(k == 1))
                            nc.scalar.activation(out=shifts[0:mr, 0:n, t], in_=ps[0:mr, 0:n], func=AF.Copy)
                        mx = cb_.tile([128, 512], dt.float32, tag="mx")
                        nc.vector.reduce_max(out=mx[0:mr, 0:n], in_=shifts[0:mr, 0:n, :],
                                             axis=mybir.AxisListType.X)
                        nc.vector.tensor_scalar(out=xt[mi][0:mr, c0:c1], in0=mx[0:mr, 0:n],
                                                scalar1=convb[0:mr, mi:mi + 1], scalar2=None, op0=ALU.add)
                wembT = cp.tile([50, NTOK], dt.float32)
                for i in range(NTOK // 128):
                    idx_t = cb_.tile([128, 1], dt.int32, tag="idx")
                    nc.sync.dma_start(out=idx_t[:], in_=d_tok.ap()[i * 128:(i + 1) * 128, :])
                    gt = cb_.tile([128, 50], dt.float32, tag="gt")
                    nc.gpsimd.indirect_dma_start(out=gt[:], out_offset=None, in_=d_wemb.ap(),
                                                 in_offset=bass.IndirectOffsetOnAxis(ap=idx_t[:, :1], axis=0))
                    pt = pp.tile([50, 128], dt.float32, tag="wtr")
                    nc.tensor.transpose(out=pt[:], in_=gt[:], identity=ident_f[:])
                    nc.scalar.activation(out=wembT[:, i * 128:(i + 1) * 128], in_=pt[:], func=AF.Copy)
                wembT_bf = cp.tile([50, NTOK], dt.bfloat16)
                nc.vector.tensor_copy(out=wembT_bf[:], in_=wembT[:])
                nc.sync.dma_start(out=xt[3][78:128, :], in_=wembT_bf[:])

            # =========== PHASE B: highway ===========
            with tc.tile_pool(name="hwp", bufs=1) as hp, \
                 tc.tile_pool(name="hwt", bufs=3) as ht, \
                 tc.tile_pool(name="psB", bufs=3, space="PSUM") as pp:
                for i in range(2):
                    xin, xout = xs[i % 2], xs[(i + 1) % 2]
                    wl = hp.tile([128, 4, 512], dt.bfloat16, tag="wl")
                    wg = hp.tile([128, 4, 512], dt.bfloat16, tag="wg")
                    bl = hp.tile([128, 4], dt.float32, tag="bl")
                    bg = hp.tile([128, 4], dt.float32, tag="bg")
                    nc.sync.dma_start(out=wl[:], in_=d_hw[f'lin{i}'].ap())
                    nc.sync.dma_start(out=wg[:], in_=d_hw[f'gate{i}'].ap())
                    nc.sync.dma_start(out=bl[:], in_=d_hw[f'lin{i}_b'].ap())
                    nc.sync.dma_start(out=bg[:], in_=d_hw[f'gate{i}_b'].ap())
                    for (c0, c1) in CHUNKS:
                        n = c1 - c0
                        for m in range(4):
                            ph = pp.tile([128, 512], dt.float32, tag="ph")
                            pg = pp.tile([128, 512], dt.float32, tag="pg")
                            for k in range(4):
                                nc.tensor.matmul(ph[:, 0:n], lhsT=wl[:, k, m * 128:(m + 1) * 128],
                                                 rhs=xin[k][:, c0:c1], start=(k == 0), stop=(k == 3))
                            for k in range(4):
                                nc.tensor.matmul(pg[:, 0:n], lhsT=wg[:, k, m * 128:(m + 1) * 128],
                                                 rhs=xin[k][:, c0:c1], start=(k == 0), stop=(k == 3))
                            hb = ht.tile([128, 512], dt.float32, tag="hb")
                            gb = ht.tile([128, 512], dt.float32, tag="gb")
                            nc.scalar.activation(out=hb[:, 0:n], in_=ph[:, 0:n], func=AF.Relu, bias=bl[:, m:m + 1])
                            nc.scalar.activation(out=gb[:, 0:n], in_=pg[:, 0:n], func=AF.Sigmoid, bias=bg[:, m:m + 1])
                            nc.vector.tensor_tensor(out=hb[:, 0:n], in0=hb[:, 0:n], in1=xin[m][:, c0:c1], op=ALU.subtract)
                            nc.vector.tensor_tensor(out=hb[:, 0:n], in0=hb[:, 0:n], in1=gb[:, 0:n], op=ALU.mult)
                            nc.vector.tensor_tensor(out=xout[m][:, c0:c1], in0=hb[:, 0:n], in1=xin[m][:, c0:c1], op=ALU.add)
            if debug:
                for m in range(4):
                    nc.sync.dma_start(out=d_xhw.ap()[m], in_=xt[m][:])

            # =========== PHASE C: ctx + q BiLSTM ===========
            with tc.tile_pool(name="ctxw", bufs=1) as cw_pool, \
                 tc.tile_pool(name="xpP", bufs=1) as xp_pool, \
                 tc.tile_pool(name="lst", bufs=1) as st_pool:
                xps = {}
                with tc.tile_pool(name="psC", bufs=2, space="PSUM") as pp:
                    for d in ('f', 'b'):
                        wt = cw_pool.tile([128, 4, 1024], dt.bfloat16, tag="ipw")
                        bt = cw_pool.tile([128, 8], dt.float32, tag="ipb")
                        nc.sync.dma_start(out=wt[:], in_=d_lstm[f'ctx_ip_{d}'].ap())
                        nc.sync.dma_start(out=bt[:], in_=d_lstm[f'ctx_ipb_{d}'].ap())
                        xp_c = xp_pool.tile([128, 8, NCTX], dt.bfloat16, tag=f"xpc{d}")
                        _emit_ip(nc, pp, xt, wt, bt, xp_c, CTX_CHUNKS, 0)
                        xp_q = xp_pool.tile([128, 8, BC * TQ], dt.bfloat16, tag=f"xpq{d}")
                        _emit_ip(nc, pp, xt, wt, bt, xp_q, Q_CHUNKS, 2048)
                        xps[d] = (xp_c, xp_q)
                if debug:
                    nc.sync.dma_start(out=d_xpdbg.ap(), in_=xps['f'][0][:])
                whhf = cw_pool.tile([128, 16, 128], dt.bfloat16, tag="whhf")
                whhb = cw_pool.tile([128, 16, 128], dt.bfloat16, tag="whhb")
                nc.sync.dma_start(out=whhf[:], in_=d_lstm['ctx_whh_f'].ap())
                nc.sync.dma_start(out=whhb[:], in_=d_lstm['ctx_whh_b'].ap())
                h0 = st_pool.tile([128, 2, 2 * BC], dt.bfloat16, tag="h0")
                nc.vector.memset(h0[:], 0.0)
                with tc.tile_pool(name="psCr", bufs=1, space="PSUM") as pp:
                    xpcf = xps['f'][0][:].rearrange("p m (b t) -> p m b t", b=BC)
                    xpcb = xps['b'][0][:].rearrange("p m (b t) -> p m b t", b=BC)
                    _emit_bilstm(nc, pp, st_pool, T, xpcf, xpcb, whhf, whhb, hseq_c, h0)
                    xpqf = xps['f'][1][:].rearrange("p m (b t) -> p m b t", b=BC)
                    xpqb = xps['b'][1][:].rearrange("p m (b t) -> p m b t", b=BC)
                    _emit_bilstm(nc, pp, st_pool, TQ, xpqf, xpqb, whhf, whhb, hseq_q, h0)
        # xt pool closed here
        if debug:
            nc.sync.dma_start(out=d_cenc.ap(), in_=hseq_c[:])
            nc.sync.dma_start(out=d_qenc.ap(), in_=hseq_q[:])

        def cenc_view(k, b):
            return hseq_c[:, k % 2, (k // 2) * BC + b, :]

        def qenc_view(k, b):
            return hseq_q[:, k % 2, (k // 2) * BC + b, :]

        # =========== PHASE D: attention -> g (DRAM) ===========
        with tc.tile_pool(name="att", bufs=1) as ap_, \
             tc.tile_pool(name="attb", bufs=2) as ab, \
             tc.tile_pool(name="psD", bufs=4, space="PSUM") as pp:
            awc = ap_.tile([128, 4], dt.float32)
            awcq = ap_.tile([128, 4], dt.float32)
            awq = ap_.tile([128, 4], dt.bfloat16)
            atb = ap_.tile([1, 1], dt.float32)
            nc.sync.dma_start(out=awc[:], in_=d_awc.ap())
            nc.sync.dma_start(out=awcq[:], in_=d_awcq.ap())
            nc.sync.dma_start(out=awq[:], in_=d_awq.ap())
            nc.sync.dma_start(out=atb[:], in_=d_ab.ap())
            for b in range(BC):
                qhat = ab.tile([128, 4, TQ], dt.bfloat16, tag="qhat")
                for k in range(4):
                    nc.vector.tensor_scalar(out=qhat[:, k, :], in0=qenc_view(k, b),
                                            scalar1=awcq[:, k:k + 1], scalar2=awc[:, k:k + 1],
                                            op0=ALU.mult, op1=ALU.add)
                pq = pp.tile([1, TQ], dt.float32, tag="ps", bufs=4)
                for k in range(4):
                    nc.tensor.matmul(pq[:], lhsT=awq[:, k:k + 1], rhs=qenc_view(k, b),
                                     start=(k == 0), stop=(k == 3))
                qwq = ab.tile([1, TQ], dt.bfloat16, tag="qwqs")
                nc.scalar.activation(out=qwq[:], in_=pq[:], func=AF.Identity, bias=atb[:])
                aT = ab.tile([64, 4, 128], dt.bfloat16, tag="aT")
                mm = ab.tile([128, 4], dt.float32, tag="mm")
                for mt in range(4):
                    ps = pp.tile([128, TQ], dt.float32, tag="ps", bufs=4)
                    for k in range(4):
                        nc.tensor.matmul(ps[:], lhsT=cenc_view(k, b)[:, mt * 128:(mt + 1) * 128],
                                         rhs=qhat[:, k, :], start=(k == 0), stop=False)
                    nc.tensor.matmul(ps[:], lhsT=ones_row_bf[:], rhs=qwq[:], start=False, stop=True)
                    nc.vector.reduce_max(out=mm[:, mt:mt + 1], in_=ps[:], axis=mybir.AxisListType.X)
                    negm = ab.tile([128, 1], dt.float32, tag="negm")
                    nc.vector.tensor_scalar(out=negm[:], in0=mm[:, mt:mt + 1], scalar1=-1.0,
                                            scalar2=None, op0=ALU.mult)
                    E = ab.tile([128, TQ], dt.float32, tag="E")
                    Z = ab.tile([128, 1], dt.float32, tag="Z")
                    nc.scalar.activation(out=E[:], in_=ps[:], func=AF.Exp, bias=negm[:], accum_out=Z[:])
                    rz = ab.tile([128, 1], dt.float32, tag="rz")
                    nc.vector.reciprocal(out=rz[:], in_=Z[:])
                    arow = ab.tile([128, TQ], dt.bfloat16, tag="arow")
                    nc.vector.tensor_scalar(out=arow[:], in0=E[:], scalar1=rz[:], scalar2=None, op0=ALU.mult)
                    pt = pp.tile([64, 128], dt.bfloat16, tag="ps", bufs=4)
                    nc.tensor.transpose(out=pt[:], in_=arow[:], identity=ident_bf[:])
                    nc.scalar.activation(out=aT[:, mt, :], in_=pt[:], func=AF.Copy)
                qtok = ab.tile([64, 4, 128], dt.bfloat16, tag="qtok")
                for k in range(4):
                    pt = pp.tile([64, 128], dt.bfloat16, tag="ps", bufs=4)
                    nc.tensor.transpose(out=pt[:], in_=qenc_view(k, b), identity=ident_bf[:])
                    nc.scalar.activation(out=qtok[:, k, :], in_=pt[:], func=AF.Copy)
                for ft in range(4):
                    pc = pp.tile([128, T], dt.float32, tag="c2q", bufs=2)
                    nc.tensor.matmul(pc[:], lhsT=qtok[:, ft, :], rhs=aT[:].rearrange("q a p -> q (a p)"),
                                     start=True, stop=True)
                    g2 = ab.tile([128, T], dt.bfloat16, tag="g2")
                    nc.scalar.activation(out=g2[:], in_=pc[:], func=AF.Copy)
                    nc.sync.dma_start(out=d_g.ap()[4 + ft, :, b * T:(b + 1) * T], in_=g2[:])
                    g3 = ab.tile([128, T], dt.bfloat16, tag="g3")
                    nc.vector.tensor_tensor(out=g3[:], in0=cenc_view(ft, b), in1=pc[:], op=ALU.mult)
                    nc.sync.dma_start(out=d_g.ap()[8 + ft, :, b * T:(b + 1) * T], in_=g3[:])
                e = ab.tile([128, 4], dt.float32, tag="e")
                nc.scalar.activation(out=e[:], in_=mm[:], func=AF.Exp)
                pz = pp.tile([1, 4], dt.float32, tag="ps", bufs=4)
                nc.tensor.matmul(pz[:], lhsT=ones_col[:], rhs=e[:], start=True, stop=True)
                zsum = ab.tile([1, 1], dt.float32, tag="zs")
                nc.vector.reduce_sum(out=zsum[:], in_=pz[:], axis=mybir.AxisListType.X)
                rzb = ab.tile([1, 1], dt.float32, tag="rzb")
                nc.vector.reciprocal(out=rzb[:], in_=zsum[:])
                pzb = pp.tile([128, 1], dt.float32, tag="ps", bufs=4)
                nc.tensor.matmul(pzb[:], lhsT=ones_row_f[:], rhs=rzb[:], start=True, stop=True)
                rzc = ab.tile([128, 1], dt.float32, tag="rzc")
                nc.scalar.activation(out=rzc[:], in_=pzb[:], func=AF.Copy)
                etil = ab.tile([128, 4], dt.bfloat16, tag="etil")
                nc.vector.tensor_scalar(out=etil[:], in0=e[:], scalar1=rzc[:], scalar2=None, op0=ALU.mult)
                ctok = ab.tile([128, 4, 512], dt.bfloat16, tag="ctok")
                for tt in range(4):
                    for ft in range(4):
                        pt = pp.tile([128, 128], dt.bfloat16, tag="ps", bufs=4)
                        nc.tensor.transpose(out=pt[:], in_=cenc_view(ft, b)[:, tt * 128:(tt + 1) * 128],
                                            identity=ident_bf[:])
                        nc.scalar.activation(out=ctok[:, tt, ft * 128:(ft + 1) * 128], in_=pt[:], func=AF.Copy)
                pq2 = pp.tile([1, 512], dt.float32, tag="c2q", bufs=2)
                for tt in range(4):
                    nc.tensor.matmul(pq2[:], lhsT=etil[:, tt:tt + 1], rhs=ctok[:, tt, :],
                                     start=(tt == 0), stop=(tt == 3))
                q2cr = ab.tile([1, 512], dt.float32, tag="q2cr")
                nc.scalar.activation(out=q2cr[:], in_=pq2[:], func=AF.Copy)
                q2cT = ab.tile([128, 4], dt.float32, tag="q2cT")
                for ft in range(4):
                    pt = pp.tile([128, 1], dt.float32, tag="ps", bufs=4)
                    nc.tensor.matmul(pt[:], lhsT=q2cr[:, ft * 128:(ft + 1) * 128], rhs=ones_row_f[:, 0:1],
                                     start=True, stop=True)
                    nc.scalar.activation(out=q2cT[:, ft:ft + 1], in_=pt[:], func=AF.Copy)
                for ft in range(4):
                    nc.sync.dma_start(out=d_g.ap()[ft, :, b * T:(b + 1) * T], in_=cenc_view(ft, b))
                    g4 = ab.tile([128, T], dt.bfloat16, tag="g4")
                    nc.vector.tensor_scalar(out=g4[:], in0=cenc_view(ft, b), scalar1=q2cT[:, ft:ft + 1],
                                            scalar2=None, op0=ALU.mult)
                    nc.sync.dma_start(out=d_g.ap()[12 + ft, :, b * T:(b + 1) * T], in_=g4[:])

        def hview(hs, k):
            return hs[:, k % 2, (k // 2) * BC:(k // 2) * BC + BC, :].rearrange("p b t -> p (b t)")

        # =========== PHASE E/F/G: mod1, mod2, out ===========
        def run_stage(name, rhs_tiles, hseq_out, stream_g=False):
            with tc.tile_pool(name=f"{name}w", bufs=1) as wp, \
                 tc.tile_pool(name=f"{name}xp", bufs=1) as xpp, \
                 tc.tile_pool(name=f"{name}st", bufs=1) as stp:
                xps = {}
                with tc.tile_pool(name=f"ps{name}i", bufs=2, space="PSUM") as pp, \
                     tc.tile_pool(name=f"{name}gs", bufs=3) as gsp:
                    for d in ('f', 'b'):
                        kt = 16 if stream_g else len(rhs_tiles)
                        wt = wp.tile([128, kt, 1024], dt.bfloat16, tag="ipw")
                        bt = wp.tile([128, 8], dt.float32, tag="ipb")
                        nc.sync.dma_start(out=wt[:], in_=d_lstm[f'{name}_ip_{d}'].ap())
                        nc.sync.dma_start(out=bt[:], in_=d_lstm[f'{name}_ipb_{d}'].ap())
                        xp = xpp.tile([128, 8, NCTX], dt.bfloat16, tag=f"xp{d}")
                        if stream_g:
                            for (c0, c1) in CTX_CHUNKS:
                                pss = [pp.tile([128, 512], dt.float32, tag=f"ipm{m}", bufs=1, name=f"ipm{m}") for m in range(8)]
                                for k in range(16):
                                    gk = gsp.tile([128, 512], dt.bfloat16, tag="gk")
                                    nc.sync.dma_start(out=gk[:], in_=d_g.ap()[k, :, c0:c1])
                                    for m in range(8):
                                        nc.tensor.matmul(pss[m][:], lhsT=wt[:, k, m * 128:(m + 1) * 128],
                                                         rhs=gk[:], start=(k == 0), stop=(k == 15))
                                for m in range(8):
                                    nc.scalar.activation(out=xp[:, m, c0:c1], in_=pss[m][:],
                                                         func=AF.Identity, bias=bt[:, m:m + 1])
                        else:
                            _emit_ip(nc, pp, rhs_tiles, wt, bt, xp, CTX_CHUNKS, 0)
                        xps[d] = xp
                whhf = wp.tile([128, 16, 128], dt.bfloat16, tag="whhf")
                whhb = wp.tile([128, 16, 128], dt.bfloat16, tag="whhb")
                nc.sync.dma_start(out=whhf[:], in_=d_lstm[f'{name}_whh_f'].ap())
                nc.sync.dma_start(out=whhb[:], in_=d_lstm[f'{name}_whh_b'].ap())
                h0 = stp.tile([128, 2, 2 * BC], dt.bfloat16, tag="h0")
                nc.vector.memset(h0[:], 0.0)
                with tc.tile_pool(name=f"ps{name}r", bufs=1, space="PSUM") as pp:
                    xpf = xps['f'][:].rearrange("p m (b t) -> p m b t", b=BC)
                    xpb = xps['b'][:].rearrange("p m (b t) -> p m b t", b=BC)
                    _emit_bilstm(nc, pp, stp, T, xpf, xpb, whhf, whhb, hseq_out, h0)

        enc_es.close()  # free hseq_c/q
        m1_es = ExitStack()
        m1_pool = m1_es.enter_context(tc.tile_pool(name="m1p", bufs=1))
        hseq_m1 = m1_pool.tile([128, 2, 2 * BC, T], dt.bfloat16, tag="hseq_m1")
        run_stage('mod1', None, hseq_m1, stream_g=True)
        if debug:
            nc.sync.dma_start(out=d_m1.ap(), in_=hseq_m1[:])
        hseq_m2 = seq_pool.tile([128, 2, 2 * BC, T], dt.bfloat16, tag="hseq_m2")
        run_stage('mod2', [hview(hseq_m1, k) for k in range(4)], hseq_m2)
        m1_es.close()
        if debug:
            nc.sync.dma_start(out=d_m2.ap(), in_=hseq_m2[:])
        hseq_o = seq_pool.tile([128, 2, 2 * BC, T], dt.bfloat16, tag="hseq_o")
        run_stage('out', [hview(hseq_m2, k) for k in range(4)], hseq_o)

        # =========== PHASE H: heads ===========
        with tc.tile_pool(name="hd", bufs=1) as hd, \
             tc.tile_pool(name="hdb", bufs=3) as hb_, \
             tc.tile_pool(name="psH", bufs=2, space="PSUM") as pp:
            w = {nm: hd.tile(list(d_heads[nm].ap().shape), d_heads[nm].ap().dtype, tag=nm, name=nm) for nm in d_heads}
            for nm in d_heads:
                nc.sync.dma_start(out=w[nm][:], in_=d_heads[nm].ap())
            for (c0, c1) in CTX_CHUNKS:
                n = c1 - c0
                ps1 = pp.tile([1, 512], dt.float32, tag="h1")
                ps2 = pp.tile([1, 512], dt.float32, tag="h2")
                for k in range(16):
                    gk = hb_.tile([128, 512], dt.bfloat16, tag="gk")
                    nc.sync.dma_start(out=gk[:], in_=d_g.ap()[k, :, c0:c1])
                    nc.tensor.matmul(ps1[:, 0:n], lhsT=w["p1_gw"][:, k:k + 1], rhs=gk[:, 0:n],
                                     start=(k == 0), stop=False)
                    nc.tensor.matmul(ps2[:, 0:n], lhsT=w["p2_gw"][:, k:k + 1], rhs=gk[:, 0:n],
                                     start=(k == 0), stop=False)
                for k in range(4):
                    nc.tensor.matmul(ps1[:, 0:n], lhsT=w["p1_mw"][:, k:k + 1], rhs=hview(hseq_m2, k)[:, c0:c1],
                                     start=False, stop=(k == 3))
                    nc.tensor.matmul(ps2[:, 0:n], lhsT=w["p2_mw"][:, k:k + 1], rhs=hview(hseq_o, k)[:, c0:c1],
                                     start=False, stop=(k == 3))
                o1 = hb_.tile([1, 512], dt.float32, tag="o1")
                o2 = hb_.tile([1, 512], dt.float32, tag="o2")
                nc.scalar.activation(out=o1[:, 0:n], in_=ps1[:, 0:n], func=AF.Identity, bias=w["p1_b"][:])
                nc.scalar.activation(out=o2[:, 0:n], in_=ps2[:, 0:n], func=AF.Identity, bias=w["p2_b"][:])
                nc.sync.dma_start(out=d_p1.ap()[:, c0:c1], in_=o1[:, 0:n])
                nc.sync.dma_start(out=d_p2.ap()[:, c0:c1], in_=o2[:, 0:n])
        es.close()

    nc.compile()
    return nc


# --------------------------------------------------------------------------
# entry point
# --------------------------------------------------------------------------

_CACHE = {}


def kernel(params, c_word, q_word, c_char, q_char):
    if 'nc' not in _CACHE:
        _CACHE['nc'] = build_program(debug=False)
    nc = _CACHE['nc']
    prep = _prep_params(params)
    in_maps = [_prep_core_inputs(prep, c_word, q_word, c_char, q_char, i) for i in range(NCORES)]
    res = run_bass_kernel_spmd(nc, in_maps, core_ids=list(range(NCORES)))
    p1 = np.concatenate([np.asarray(res.results[i]["p1"], np.float32).reshape(BC, T) for i in range(NCORES)])
    p2 = np.concatenate([np.asarray(res.results[i]["p2"], np.float32).reshape(BC, T) for i in range(NCORES)])
    return p1, p2
